# revision 9
# baseline (speedup 1.0000x reference)
"""3-layer GCN (GCNConv x3) on Trainium2, 8 NeuronCores.

Strategy: dst nodes sharded across 8 cores (12500 each); edges partitioned
by (dst shard, dst group, src shard); weights replicated; between layers the
per-shard activations are AllGathered so every core gathers source features
locally.

Device layout (grouped-blocked): each core's 12500 dst nodes are assigned
round-robin (by descending degree, for load balance) to 8 groups of <=1563.
Group g lives on Q7 core g (SBUF partitions 16g..16g+15 = feature rows).
Grouped position P = g*1563 + p indexes compact [*, 12504] tensors.

Per layer, per src-shard chunk r (8 chunks of 12500 src nodes):
  - table[16k+f, 1+P_src] = dinv[s] * h[f, s] for shard r, replicated to all
    8 groups (col 0 = zero slot for padding)
  - ap_gather pulls per-edge messages along each group's edge stream
  - in-place tensor_tensor_scan cumsums the stream
  - a small ap_gather reads the cumsum at per-node segment ends; adjacent
    differences accumulate into agg[16g+f, p]
The pipeline double-buffers tables and msg/cumsum tiles so table DMA loads
and the Vector scan overlap the GPSIMD gathers (the bottleneck engine).

Transforms run post-aggregation in compact layout staged inside the (dead)
table buffer: aggc rows 0..FA-1 = dinv^2*agg, row FA = dinv; one matmul with
Waug (W rows + bias row) gives relu(dinv*(W^T y + b)) = dinv*h_next, which
is exactly the next layer's gather table (written to hsh -> AllGather).
Layer 3 is transform-first (W3 before aggregation), aggregating 2 features.
"""

import contextlib
import numpy as np

N = 100000
E0 = 3200000
CORES = 8
NSH = N // CORES            # 12500 dst nodes per core
G = 8                       # Q7 groups per core
SUB = 1563                  # group capacity (groups 0-3: 1563, 4-7: 1562)
D = G * SUB                 # 12504 grouped-position space per shard
NCHUNK = 8                  # src chunks == shards
TABW = D + 1                # table cols (slot 0 = zero)
NIB = 1568                  # ends gathered per (group, chunk): 1+SUB pad to 1568
FA1, FA2, FA3 = 3, 16, 2    # aggregation dims per layer
F1, F2 = 16, 32


def _preprocess(x, edge_index, W1, b1, W2, b2, W3, b3):
    src = np.asarray(edge_index[0], dtype=np.int64)
    dst = np.asarray(edge_index[1], dtype=np.int64)
    # deg includes the self-loop; the loop itself is handled on-device as
    # agg += dinv*h_local, so it is excluded from the gather streams
    deg = (np.bincount(dst, minlength=N) + 1).astype(np.float64)
    dinv = np.where(deg > 0, 1.0 / np.sqrt(deg), 0.0).astype(np.float32)

    # node -> (group, pos) within its shard: degree-desc round-robin
    core_of = np.arange(N) // NSH
    within = np.arange(N) % NSH
    grp = np.empty(N, np.int64)
    pos = np.empty(N, np.int64)
    for c in range(CORES):
        nodes = np.arange(c * NSH, (c + 1) * NSH)
        order = np.argsort(-deg[nodes], kind="stable")
        rank = np.empty(NSH, np.int64)
        rank[order] = np.arange(NSH)
        grp[nodes] = rank % G
        pos[nodes] = rank // G
    P = grp * SUB + pos                                  # grouped position

    # per-edge bucket + sort
    c_e = core_of[dst]
    g_e = grp[dst]
    r_e = core_of[src]
    p_e = pos[dst]
    bucket = (c_e * G + g_e) * NCHUNK + r_e              # [0, 512)
    order = np.lexsort((p_e, bucket))
    b_s = bucket[order]

    counts = np.bincount(b_s, minlength=CORES * G * NCHUNK)
    maxb = int(counts.max())
    # mult of 32 so each chunk's int16 idx slice starts 4-byte aligned
    L_c = ((maxb + 1 + 31) // 32) * 32
    L_c = max(L_c, 64)

    starts = np.zeros(CORES * G * NCHUNK, dtype=np.int64)
    starts[1:] = np.cumsum(counts)[:-1]
    slot = np.arange(len(b_s)) - starts[b_s] + 1         # 1-based

    idx_main = np.zeros((CORES, G, NCHUNK, L_c), dtype=np.int16)
    idx_val = (1 + P[src][order]).astype(np.int16)
    idx_main[c_e[order], g_e[order], r_e[order], slot] = idx_val

    ends = np.zeros((CORES, G, NCHUNK, SUB), dtype=np.int64)
    np.maximum.at(ends, (c_e[order], g_e[order], r_e[order], p_e[order]), slot)
    ends = np.maximum.accumulate(ends, axis=3)
    ends_seq = np.zeros((CORES, G, NCHUNK, NIB), dtype=np.int16)
    ends_seq[:, :, :, 1:1 + SUB] = ends
    ends_seq[:, :, :, 1 + SUB:] = ends[:, :, :, -1:]

    def wrap16(a):
        sh = a.shape
        return a.reshape(*sh[:-1], sh[-1] // 16, 16).swapaxes(-1, -2)

    idx_main_w = wrap16(idx_main.reshape(CORES, G, NCHUNK * L_c))
    idx_main_w = idx_main_w.reshape(CORES, G * 16, NCHUNK * L_c // 16)
    ends_w = wrap16(ends_seq.reshape(CORES, G, NCHUNK * NIB))
    ends_w = ends_w.reshape(CORES, G * 16, NCHUNK * NIB // 16)

    # xtab bf16 [3, NCHUNK*D]: shard r block, grouped order; pads zero
    import ml_dtypes
    xs = (x.astype(np.float32) * dinv[:, None])          # [N, 3]
    xtab = np.zeros((3, NCHUNK * D), dtype=ml_dtypes.bfloat16)
    cols = core_of * D + P
    xtab[:, cols] = xs.T.astype(ml_dtypes.bfloat16)

    xself = np.ascontiguousarray(xtab.reshape(3, NCHUNK, D).swapaxes(0, 1))

    # per-core compact dinv [1, D] (pads zero)
    dinv_g = np.zeros((CORES, 1, D), dtype=np.float32)
    dinv_g[core_of, 0, P] = dinv

    W1aug = np.zeros((4, 16), dtype=np.float32)
    W1aug[:3] = W1.astype(np.float32)
    W1aug[3] = b1.astype(np.float32)
    W2aug = np.zeros((17, 32), dtype=np.float32)
    W2aug[:16] = W2.astype(np.float32)
    W2aug[16] = b2.astype(np.float32)
    b3rep = np.zeros((128, 1), dtype=np.float32)
    for g in range(G):
        b3rep[16 * g:16 * g + 2, 0] = b3.astype(np.float32)

    host = dict(
        L_c=L_c,
        idx_main_w=idx_main_w, ends_w=ends_w,
        xtab=xtab, xself=xself, dinv_g=dinv_g,
        W1aug=W1aug, W2aug=W2aug,
        W3=np.ascontiguousarray(W3.astype(np.float32)),
        b3rep=b3rep,
        P=P, core_of=core_of,
    )
    return host


def _build_bass(L_c):
    import concourse.bacc as bacc
    import concourse.mybir as mybir
    from concourse import tile

    dt = mybir.dt
    f32 = dt.float32
    bf16 = dt.bfloat16
    Alu = mybir.AluOpType
    Act = mybir.ActivationFunctionType

    nc = bacc.Bacc("TRN2", target_bir_lowering=False, debug=False,
                   num_devices=CORES)

    LQ = NCHUNK * L_c // 16
    EQ = NCHUNK * NIB // 16

    idx_main_d = nc.dram_tensor("idx_main", [128, LQ], dt.int16, kind="ExternalInput").ap()
    idx_ends_d = nc.dram_tensor("idx_ends", [128, EQ], dt.int16, kind="ExternalInput").ap()
    xtab_d = nc.dram_tensor("xtab", [3, NCHUNK * D], bf16, kind="ExternalInput").ap()
    xself_d = nc.dram_tensor("xself", [3, D], bf16, kind="ExternalInput").ap()
    dinv_d = nc.dram_tensor("dinv_g", [1, D], f32, kind="ExternalInput").ap()
    W1_d = nc.dram_tensor("W1aug", [4, 16], f32, kind="ExternalInput").ap()
    W2_d = nc.dram_tensor("W2aug", [17, 32], f32, kind="ExternalInput").ap()
    W3_d = nc.dram_tensor("W3", [32, 2], f32, kind="ExternalInput").ap()
    b3_d = nc.dram_tensor("b3rep", [128, 1], f32, kind="ExternalInput").ap()
    out_d = nc.dram_tensor("out", [2, D], f32, kind="ExternalOutput").ap()

    hsh2 = nc.dram_tensor("hsh2", [16, D], f32).ap()
    hga2 = nc.dram_tensor("hga2", [128, D], f32, addr_space="Shared").ap()
    hsh3 = nc.dram_tensor("hsh3", [2, D], f32).ap()
    hga3 = nc.dram_tensor("hga3", [16, D], f32, addr_space="Shared").ap()

    with tile.TileContext(nc) as tc, contextlib.ExitStack() as ctx:
        pool = ctx.enter_context(tc.tile_pool(name="main", bufs=1))
        psum = ctx.enter_context(tc.tile_pool(name="ps", bufs=4, space="PSUM"))

        idxm = pool.tile([128, LQ], dt.int16, tag="idxm")
        nc.gpsimd.dma_start(idxm[:, :], idx_main_d[:, :])
        idxe = pool.tile([128, EQ], dt.int16, tag="idxe")
        nc.gpsimd.dma_start(idxe[:, :], idx_ends_d[:, :])

        dinv128 = pool.tile([128, SUB], f32, tag="dv")
        for g in range(G):
            bc = dinv_d[0:1, g * SUB:(g + 1) * SUB].partition_broadcast(16)
            nc.gpsimd.dma_start(dinv128[16 * g:16 * (g + 1), 0:SUB], bc)
        dinvsq = pool.tile([128, SUB], f32, tag="dvsq")
        nc.vector.tensor_tensor(dinvsq[:, :], dinv128[:, :], dinv128[:, :],
                                Alu.mult)

        W1t = pool.tile([4, 16], f32, tag="w1")
        nc.gpsimd.dma_start(W1t[:, :], W1_d[:, :])
        W2t = pool.tile([17, 32], f32, tag="w2")
        nc.gpsimd.dma_start(W2t[:, :], W2_d[:, :])
        W3t = pool.tile([64, 2], f32, tag="w3")
        nc.gpsimd.dma_start(W3t[32:64, :], W3_d[:, :])
        b3t = pool.tile([128, 1], f32, tag="b3")
        nc.gpsimd.dma_start(b3t[:, :], b3_d[:, :])

        tblA = pool.tile([128, TABW], f32, tag="tblA")
        tblB = pool.tile([128, TABW], f32, tag="tblB")
        tbl = [tblA, tblB]
        mcA = pool.tile([128, L_c], f32, tag="mcA")
        mcB = pool.tile([128, L_c], f32, tag="mcB")
        mc = [mcA, mcB]
        endsgA = pool.tile([128, NIB], f32, tag="endsA")
        endsgB = pool.tile([128, NIB], f32, tag="endsB")
        endsgs = [endsgA, endsgB]
        selftab = pool.tile([128, SUB], f32, tag="selftab")
        agg = pool.tile([128, SUB], f32, tag="agg")
        dif = pool.tile([128, SUB], f32, tag="dif")

        nc.vector.memset(tbl[0][:, :], 0.0)
        nc.vector.memset(tbl[1][:, :], 0.0)

        def table_load(lay, r):
            tb = tbl[r % 2]
            if lay == 1:
                for k in range(G):
                    nc.gpsimd.dma_start(tb[16 * k:16 * k + 3, 1:TABW],
                                        xtab_d[0:3, r * D:(r + 1) * D])
            elif lay == 2:
                for k in range(G):
                    nc.gpsimd.dma_start(tb[16 * k:16 * k + 16, 1:TABW],
                                        hga2[16 * r:16 * r + 16, :])
            else:
                for k in range(G):
                    nc.gpsimd.dma_start(tb[16 * k:16 * k + 2, 1:TABW],
                                        hga3[2 * r:2 * r + 2, :])

        def main_gather(r):
            nc.gpsimd.ap_gather(
                mc[r % 2][:, :], tbl[r % 2][:, :],
                idxm[:, r * (L_c // 16):(r + 1) * (L_c // 16)],
                channels=128, num_elems=TABW, d=1, num_idxs=L_c)

        def aggregate(lay):
            table_load(lay, 0)
            for g in range(G):
                s = slice(g * SUB, (g + 1) * SUB)
                if lay == 1:
                    nc.gpsimd.dma_start(selftab[16 * g:16 * g + 3, 0:SUB],
                                        xself_d[0:3, s])
                elif lay == 2:
                    nc.gpsimd.dma_start(selftab[16 * g:16 * g + 16, 0:SUB],
                                        hsh2[0:16, s])
                else:
                    nc.gpsimd.dma_start(selftab[16 * g:16 * g + 2, 0:SUB],
                                        hsh3[0:2, s])
            main_gather(0)
            for r in range(NCHUNK):
                nc.vector.tensor_tensor_scan(
                    mc[r % 2][:, :], mc[r % 2][:, :], mc[r % 2][:, :], 0.0,
                    Alu.add, Alu.bypass)
                if r + 1 < NCHUNK:
                    table_load(lay, r + 1)
                    main_gather(r + 1)
                endsg = endsgs[r % 2]
                nc.gpsimd.ap_gather(
                    endsg[:, :], mc[r % 2][:, :],
                    idxe[:, r * (NIB // 16):(r + 1) * (NIB // 16)],
                    channels=128, num_elems=L_c, d=1, num_idxs=NIB)
                if r == 0:
                    nc.vector.tensor_tensor(
                        agg[:, :], endsg[:, 1:1 + SUB], endsg[:, 0:SUB],
                        Alu.subtract)
                else:
                    nc.vector.tensor_tensor(
                        dif[:, :], endsg[:, 1:1 + SUB], endsg[:, 0:SUB],
                        Alu.subtract)
                    nc.vector.tensor_tensor(
                        agg[:, :], agg[:, :], dif[:, :], Alu.add)
            nc.vector.tensor_tensor(agg[:, :], agg[:, :], selftab[:, :],
                                    Alu.add)

        def transform(lay):
            # aggc rows 0:FA = dinv^2*agg (compact), row FA = dinv
            # sloc rows 32:32+Fout = relu(Waug^T @ aggc) = dinv*h_next
            FA = FA1 if lay == 1 else FA2
            Fout = F1 if lay == 1 else F2
            Wt = W1t if lay == 1 else W2t
            tb = tbl[0]
            nc.vector.tensor_tensor(agg[:, :], agg[:, :], dinvsq[:, :],
                                    Alu.mult)
            for g in range(G):
                nc.gpsimd.dma_start(tb[0:FA, g * SUB:(g + 1) * SUB],
                                    agg[16 * g:16 * g + FA, 0:SUB])
            nc.gpsimd.dma_start(tb[FA:FA + 1, 0:D], dinv_d[:, :])
            for t0 in range(0, D, 512):
                w = min(512, D - t0)
                pt = psum.tile([Fout, 512], f32, tag="pt")
                nc.tensor.matmul(pt[0:Fout, 0:w], Wt[0:FA + 1, 0:Fout],
                                 tb[0:FA + 1, t0:t0 + w], start=True, stop=True)
                nc.scalar.activation(tb[32:32 + Fout, t0:t0 + w],
                                     pt[0:Fout, 0:w], Act.Relu)
            if lay == 1:
                nc.gpsimd.dma_start(hsh2[:, :], tb[32:48, 0:D])
                nc.gpsimd.collective_compute(
                    "AllGather", mybir.AluOpType.bypass,
                    replica_groups=[list(range(CORES))],
                    ins=[hsh2[:, :]], outs=[hga2[:, :]])
            else:
                for t0 in range(0, D, 512):
                    w = min(512, D - t0)
                    pt = psum.tile([2, 512], f32, tag="pt3")
                    nc.tensor.matmul(pt[0:2, 0:w], W3t[32:64, 0:2],
                                     tb[32:64, t0:t0 + w], start=True, stop=True)
                    nc.scalar.activation(tb[64:66, t0:t0 + w], pt[0:2, 0:w],
                                         Act.Identity)
                nc.gpsimd.dma_start(hsh3[:, :], tb[64:66, 0:D])
                nc.gpsimd.collective_compute(
                    "AllGather", mybir.AluOpType.bypass,
                    replica_groups=[list(range(CORES))],
                    ins=[hsh3[:, :]], outs=[hga3[:, :]])

        # ---- layer 1 ----
        aggregate(1)
        transform(1)
        nc.vector.memset(tbl[0][:, 0:1], 0.0)   # re-zero slot col dirtied by aggc

        # ---- layer 2 ----
        aggregate(2)
        transform(2)
        # layer-3 tables only fill rows 16k+0:2 -> full re-zero of both bufs
        nc.vector.memset(tbl[0][:, :], 0.0)
        nc.vector.memset(tbl[1][:, :], 0.0)

        # ---- layer 3 ----
        aggregate(3)
        nc.vector.tensor_tensor(agg[:, :], agg[:, :], dinv128[:, :], Alu.mult)
        nc.scalar.activation(agg[:, :], agg[:, :], Act.Identity,
                             bias=b3t[:, 0:1], scale=1.0)
        for g in range(G):
            nc.gpsimd.dma_start(out_d[0:2, g * SUB:(g + 1) * SUB],
                                agg[16 * g:16 * g + 2, 0:SUB])

    nc.compile()
    return nc


_CACHED = {}


def _in_maps(host):
    maps = []
    for c in range(CORES):
        maps.append({
            "idx_main": np.ascontiguousarray(host["idx_main_w"][c]),
            "idx_ends": np.ascontiguousarray(host["ends_w"][c]),
            "xtab": host["xtab"],
            "xself": np.ascontiguousarray(host["xself"][c]),
            "dinv_g": host["dinv_g"][c],
            "W1aug": host["W1aug"], "W2aug": host["W2aug"],
            "W3": host["W3"], "b3rep": host["b3rep"],
        })
    return maps


def kernel(**inputs):
    x = np.asarray(inputs["x"])
    edge_index = np.asarray(inputs["edge_index"])
    host = _preprocess(x, edge_index, inputs["W1"], inputs["b1"],
                       inputs["W2"], inputs["b2"], inputs["W3"], inputs["b3"])
    L_c = host["L_c"]
    if L_c not in _CACHED:
        _CACHED[L_c] = _build_bass(L_c)
    nc = _CACHED[L_c]

    from concourse.bass_utils import run_bass_kernel_spmd
    res = run_bass_kernel_spmd(nc, _in_maps(host), core_ids=list(range(CORES)))

    out = np.empty((N, 2), dtype=np.float32)
    P, core_of = host["P"], host["core_of"]
    for c in range(CORES):
        nodes = np.arange(c * NSH, (c + 1) * NSH)
        out[nodes] = res.results[c]["out"][:, P[nodes]].T
    return out


# revision 11
# speedup vs baseline: 1.0651x; 1.0651x over previous
"""3-layer GCN (GCNConv x3) on Trainium2, 8 NeuronCores.

Strategy: dst nodes sharded across 8 cores (12500 each); edges partitioned
by (dst shard, dst group, src shard); weights replicated; between layers the
per-shard activations are AllGathered so every core gathers source features
locally.

Device layout (grouped-blocked): each core's 12500 dst nodes are assigned
round-robin (by descending degree, for load balance) to 8 groups of <=1563.
Group g lives on Q7 core g (SBUF partitions 16g..16g+15 = feature rows).
Grouped position P = g*1563 + p indexes compact [*, 12504] tensors.

Per layer, per src-shard chunk r (8 chunks of 12500 src nodes):
  - table[16k+f, 1+P_src] = dinv[s] * h[f, s] for shard r, replicated to all
    8 groups (col 0 = zero slot for padding)
  - ap_gather pulls per-edge messages along each group's edge stream
  - in-place tensor_tensor_scan cumsums the stream
  - a small ap_gather reads the cumsum at per-node segment ends; adjacent
    differences accumulate into agg[16g+f, p]
The pipeline double-buffers tables and msg/cumsum tiles so table DMA loads
and the Vector scan overlap the GPSIMD gathers (the bottleneck engine).

Transforms run post-aggregation in compact layout staged inside the (dead)
table buffer: aggc rows 0..FA-1 = dinv^2*agg, row FA = dinv; one matmul with
Waug (W rows + bias row) gives relu(dinv*(W^T y + b)) = dinv*h_next, which
is exactly the next layer's gather table (written to hsh -> AllGather).
Layer 3 is transform-first (W3 before aggregation), aggregating 2 features.
"""

import contextlib
import numpy as np

N = 100000
E0 = 3200000
CORES = 8
NSH = N // CORES            # 12500 dst nodes per core
G = 8                       # Q7 groups per core
SUB = 1563                  # group capacity (groups 0-3: 1563, 4-7: 1562)
D = G * SUB                 # 12504 grouped-position space per shard
NCHUNK = 8                  # src chunks == shards
TABW = D + 1                # table cols (slot 0 = zero)
NIB = 1568                  # ends gathered per (group, chunk): 1+SUB pad to 1568
FA1, FA2, FA3 = 3, 16, 2    # aggregation dims per layer
F1, F2 = 16, 32


def _preprocess(x, edge_index, W1, b1, W2, b2, W3, b3):
    src = np.asarray(edge_index[0], dtype=np.int64)
    dst = np.asarray(edge_index[1], dtype=np.int64)
    # deg includes the self-loop; the loop itself is handled on-device as
    # agg += dinv*h_local, so it is excluded from the gather streams
    deg = (np.bincount(dst, minlength=N) + 1).astype(np.float64)
    dinv = np.where(deg > 0, 1.0 / np.sqrt(deg), 0.0).astype(np.float32)

    # node -> (group, pos) within its shard: degree-desc round-robin
    core_of = np.arange(N) // NSH
    within = np.arange(N) % NSH
    grp = np.empty(N, np.int64)
    pos = np.empty(N, np.int64)
    for c in range(CORES):
        nodes = np.arange(c * NSH, (c + 1) * NSH)
        order = np.argsort(-deg[nodes], kind="stable")
        rank = np.empty(NSH, np.int64)
        rank[order] = np.arange(NSH)
        grp[nodes] = rank % G
        pos[nodes] = rank // G
    P = grp * SUB + pos                                  # grouped position

    # per-edge bucket + sort
    c_e = core_of[dst]
    g_e = grp[dst]
    r_e = core_of[src]
    p_e = pos[dst]
    bucket = (c_e * G + g_e) * NCHUNK + r_e              # [0, 512)
    order = np.lexsort((p_e, bucket))
    b_s = bucket[order]

    counts = np.bincount(b_s, minlength=CORES * G * NCHUNK)
    maxb = int(counts.max())
    # mult of 32 so each chunk's int16 idx slice starts 4-byte aligned
    L_c = ((maxb + 1 + 31) // 32) * 32
    L_c = max(L_c, 64)

    starts = np.zeros(CORES * G * NCHUNK, dtype=np.int64)
    starts[1:] = np.cumsum(counts)[:-1]
    slot = np.arange(len(b_s)) - starts[b_s] + 1         # 1-based

    idx_main = np.zeros((CORES, G, NCHUNK, L_c), dtype=np.int16)
    idx_val = (1 + P[src][order]).astype(np.int16)
    idx_main[c_e[order], g_e[order], r_e[order], slot] = idx_val

    ends = np.zeros((CORES, G, NCHUNK, SUB), dtype=np.int64)
    np.maximum.at(ends, (c_e[order], g_e[order], r_e[order], p_e[order]), slot)
    ends = np.maximum.accumulate(ends, axis=3)
    ends_seq = np.zeros((CORES, G, NCHUNK, NIB), dtype=np.int16)
    ends_seq[:, :, :, 1:1 + SUB] = ends
    ends_seq[:, :, :, 1 + SUB:] = ends[:, :, :, -1:]

    def wrap16(a):
        sh = a.shape
        return a.reshape(*sh[:-1], sh[-1] // 16, 16).swapaxes(-1, -2)

    idx_main_w = wrap16(idx_main.reshape(CORES, G, NCHUNK * L_c))
    idx_main_w = idx_main_w.reshape(CORES, G * 16, NCHUNK * L_c // 16)
    ends_w = wrap16(ends_seq.reshape(CORES, G, NCHUNK * NIB))
    ends_w = ends_w.reshape(CORES, G * 16, NCHUNK * NIB // 16)

    # xtab bf16 [3, NCHUNK*D]: shard r block, grouped order; pads zero
    import ml_dtypes
    xs = (x.astype(np.float32) * dinv[:, None])          # [N, 3]
    xtab = np.zeros((3, NCHUNK * D), dtype=ml_dtypes.bfloat16)
    cols = core_of * D + P
    xtab[:, cols] = xs.T.astype(ml_dtypes.bfloat16)

    xself = np.ascontiguousarray(xtab.reshape(3, NCHUNK, D).swapaxes(0, 1))

    # per-core compact dinv [1, D] (pads zero)
    dinv_g = np.zeros((CORES, 1, D), dtype=np.float32)
    dinv_g[core_of, 0, P] = dinv

    W1aug = np.zeros((4, 16), dtype=np.float32)
    W1aug[:3] = W1.astype(np.float32)
    W1aug[3] = b1.astype(np.float32)
    W2aug = np.zeros((17, 32), dtype=np.float32)
    W2aug[:16] = W2.astype(np.float32)
    W2aug[16] = b2.astype(np.float32)
    b3rep = np.zeros((128, 1), dtype=np.float32)
    for g in range(G):
        b3rep[16 * g:16 * g + 2, 0] = b3.astype(np.float32)

    host = dict(
        L_c=L_c,
        idx_main_w=idx_main_w, ends_w=ends_w,
        xtab=xtab, xself=xself, dinv_g=dinv_g,
        W1aug=W1aug, W2aug=W2aug,
        W3=np.ascontiguousarray(W3.astype(np.float32)),
        b3rep=b3rep,
        P=P, core_of=core_of,
    )
    return host


def _build_bass(L_c):
    import concourse.bacc as bacc
    import concourse.mybir as mybir
    from concourse import tile

    dt = mybir.dt
    f32 = dt.float32
    bf16 = dt.bfloat16
    Alu = mybir.AluOpType
    Act = mybir.ActivationFunctionType

    nc = bacc.Bacc("TRN2", target_bir_lowering=False, debug=False,
                   num_devices=CORES)

    LQ = NCHUNK * L_c // 16
    EQ = NCHUNK * NIB // 16

    idx_main_d = nc.dram_tensor("idx_main", [128, LQ], dt.int16, kind="ExternalInput").ap()
    idx_ends_d = nc.dram_tensor("idx_ends", [128, EQ], dt.int16, kind="ExternalInput").ap()
    xself_d = nc.dram_tensor("xself", [3, D], bf16, kind="ExternalInput").ap()
    dinv_d = nc.dram_tensor("dinv_g", [1, D], f32, kind="ExternalInput").ap()
    W1_d = nc.dram_tensor("W1aug", [4, 16], f32, kind="ExternalInput").ap()
    W2_d = nc.dram_tensor("W2aug", [17, 32], f32, kind="ExternalInput").ap()
    W3_d = nc.dram_tensor("W3", [32, 2], f32, kind="ExternalInput").ap()
    b3_d = nc.dram_tensor("b3rep", [128, 1], f32, kind="ExternalInput").ap()
    out_d = nc.dram_tensor("out", [2, D], f32, kind="ExternalOutput").ap()

    xsh = nc.dram_tensor("xsh", [3, D], bf16).ap()
    xga = nc.dram_tensor("xga", [3 * CORES, D], bf16, addr_space="Shared").ap()
    hsh2 = nc.dram_tensor("hsh2", [16, D], f32).ap()
    hga2 = nc.dram_tensor("hga2", [128, D], f32, addr_space="Shared").ap()
    hsh3 = nc.dram_tensor("hsh3", [2, D], f32).ap()
    hga3 = nc.dram_tensor("hga3", [16, D], f32, addr_space="Shared").ap()

    with tile.TileContext(nc) as tc, contextlib.ExitStack() as ctx:
        pool = ctx.enter_context(tc.tile_pool(name="main", bufs=1))
        psum = ctx.enter_context(tc.tile_pool(name="ps", bufs=4, space="PSUM"))

        nc.gpsimd.dma_start(xsh[:, :], xself_d[:, :])
        nc.gpsimd.collective_compute(
            "AllGather", mybir.AluOpType.bypass,
            replica_groups=[list(range(CORES))],
            ins=[xsh[:, :]], outs=[xga[:, :]])

        idxm = pool.tile([128, LQ], dt.int16, tag="idxm")
        nc.gpsimd.dma_start(idxm[:, :], idx_main_d[:, :])
        idxe = pool.tile([128, EQ], dt.int16, tag="idxe")
        nc.gpsimd.dma_start(idxe[:, :], idx_ends_d[:, :])

        dinv128 = pool.tile([128, SUB], f32, tag="dv")
        for g in range(G):
            bc = dinv_d[0:1, g * SUB:(g + 1) * SUB].partition_broadcast(16)
            nc.gpsimd.dma_start(dinv128[16 * g:16 * (g + 1), 0:SUB], bc)
        dinvsq = pool.tile([128, SUB], f32, tag="dvsq")
        nc.vector.tensor_tensor(dinvsq[:, :], dinv128[:, :], dinv128[:, :],
                                Alu.mult)

        W1t = pool.tile([4, 16], f32, tag="w1")
        nc.gpsimd.dma_start(W1t[:, :], W1_d[:, :])
        W2t = pool.tile([17, 32], f32, tag="w2")
        nc.gpsimd.dma_start(W2t[:, :], W2_d[:, :])
        W3t = pool.tile([64, 2], f32, tag="w3")
        nc.gpsimd.dma_start(W3t[32:64, :], W3_d[:, :])
        b3t = pool.tile([128, 1], f32, tag="b3")
        nc.gpsimd.dma_start(b3t[:, :], b3_d[:, :])

        tblA = pool.tile([128, TABW], f32, tag="tblA")
        tblB = pool.tile([128, TABW], f32, tag="tblB")
        tbl = [tblA, tblB]
        mcA = pool.tile([128, L_c], f32, tag="mcA")
        mcB = pool.tile([128, L_c], f32, tag="mcB")
        mc = [mcA, mcB]
        endsgA = pool.tile([128, NIB], f32, tag="endsA")
        endsgB = pool.tile([128, NIB], f32, tag="endsB")
        endsgs = [endsgA, endsgB]
        selftab = pool.tile([128, SUB], f32, tag="selftab")
        agg = pool.tile([128, SUB], f32, tag="agg")
        dif = pool.tile([128, SUB], f32, tag="dif")

        nc.vector.memset(tbl[0][:, :], 0.0)
        nc.vector.memset(tbl[1][:, :], 0.0)

        def table_load(lay, r):
            tb = tbl[r % 2]
            if lay == 1:
                for k in range(G):
                    nc.gpsimd.dma_start(tb[16 * k:16 * k + 3, 1:TABW],
                                        xga[3 * r:3 * r + 3, :])
            elif lay == 2:
                for k in range(G):
                    nc.sync.dma_start(tb[16 * k:16 * k + 16, 1:TABW],
                                      hga2[16 * r:16 * r + 16, :])
            else:
                for k in range(G):
                    nc.sync.dma_start(tb[16 * k:16 * k + 2, 1:TABW],
                                      hga3[2 * r:2 * r + 2, :])

        def main_gather(r):
            nc.gpsimd.ap_gather(
                mc[r % 2][:, :], tbl[r % 2][:, :],
                idxm[:, r * (L_c // 16):(r + 1) * (L_c // 16)],
                channels=128, num_elems=TABW, d=1, num_idxs=L_c)

        def aggregate(lay):
            table_load(lay, 0)
            for g in range(G):
                s = slice(g * SUB, (g + 1) * SUB)
                if lay == 1:
                    nc.gpsimd.dma_start(selftab[16 * g:16 * g + 3, 0:SUB],
                                        xself_d[0:3, s])
                elif lay == 2:
                    nc.sync.dma_start(selftab[16 * g:16 * g + 16, 0:SUB],
                                      hsh2[0:16, s])
                else:
                    nc.sync.dma_start(selftab[16 * g:16 * g + 2, 0:SUB],
                                      hsh3[0:2, s])
            main_gather(0)
            for r in range(NCHUNK):
                nc.vector.tensor_tensor_scan(
                    mc[r % 2][:, :], mc[r % 2][:, :], mc[r % 2][:, :], 0.0,
                    Alu.add, Alu.bypass)
                if r + 1 < NCHUNK:
                    table_load(lay, r + 1)
                    main_gather(r + 1)
                endsg = endsgs[r % 2]
                nc.gpsimd.ap_gather(
                    endsg[:, :], mc[r % 2][:, :],
                    idxe[:, r * (NIB // 16):(r + 1) * (NIB // 16)],
                    channels=128, num_elems=L_c, d=1, num_idxs=NIB)
                if r == 0:
                    nc.vector.tensor_tensor(
                        agg[:, :], endsg[:, 1:1 + SUB], endsg[:, 0:SUB],
                        Alu.subtract)
                else:
                    nc.vector.tensor_tensor(
                        dif[:, :], endsg[:, 1:1 + SUB], endsg[:, 0:SUB],
                        Alu.subtract)
                    nc.vector.tensor_tensor(
                        agg[:, :], agg[:, :], dif[:, :], Alu.add)
            nc.vector.tensor_tensor(agg[:, :], agg[:, :], selftab[:, :],
                                    Alu.add)

        def transform(lay):
            # aggc rows 0:FA = dinv^2*agg (compact), row FA = dinv
            # sloc rows 32:32+Fout = relu(Waug^T @ aggc) = dinv*h_next
            FA = FA1 if lay == 1 else FA2
            Fout = F1 if lay == 1 else F2
            Wt = W1t if lay == 1 else W2t
            tb = tbl[0]
            nc.vector.tensor_tensor(agg[:, :], agg[:, :], dinvsq[:, :],
                                    Alu.mult)
            for g in range(G):
                nc.sync.dma_start(tb[0:FA, g * SUB:(g + 1) * SUB],
                                  agg[16 * g:16 * g + FA, 0:SUB])
            nc.sync.dma_start(tb[FA:FA + 1, 0:D], dinv_d[:, :])
            for t0 in range(0, D, 512):
                w = min(512, D - t0)
                pt = psum.tile([Fout, 512], f32, tag="pt")
                nc.tensor.matmul(pt[0:Fout, 0:w], Wt[0:FA + 1, 0:Fout],
                                 tb[0:FA + 1, t0:t0 + w], start=True, stop=True)
                nc.scalar.activation(tb[32:32 + Fout, t0:t0 + w],
                                     pt[0:Fout, 0:w], Act.Relu)
            if lay == 1:
                nc.sync.dma_start(hsh2[:, :], tb[32:48, 0:D])
                nc.gpsimd.collective_compute(
                    "AllGather", mybir.AluOpType.bypass,
                    replica_groups=[list(range(CORES))],
                    ins=[hsh2[:, :]], outs=[hga2[:, :]])
            else:
                for t0 in range(0, D, 512):
                    w = min(512, D - t0)
                    pt = psum.tile([2, 512], f32, tag="pt3")
                    nc.tensor.matmul(pt[0:2, 0:w], W3t[32:64, 0:2],
                                     tb[32:64, t0:t0 + w], start=True, stop=True)
                    nc.scalar.activation(tb[64:66, t0:t0 + w], pt[0:2, 0:w],
                                         Act.Identity)
                nc.sync.dma_start(hsh3[:, :], tb[64:66, 0:D])
                nc.gpsimd.collective_compute(
                    "AllGather", mybir.AluOpType.bypass,
                    replica_groups=[list(range(CORES))],
                    ins=[hsh3[:, :]], outs=[hga3[:, :]])

        # ---- layer 1 ----
        aggregate(1)
        transform(1)
        nc.vector.memset(tbl[0][:, 0:1], 0.0)   # re-zero slot col dirtied by aggc

        # ---- layer 2 ----
        aggregate(2)
        transform(2)
        # layer-3 tables only fill rows 16k+0:2 -> full re-zero of both bufs
        nc.vector.memset(tbl[0][:, :], 0.0)
        nc.vector.memset(tbl[1][:, :], 0.0)

        # ---- layer 3 ----
        aggregate(3)
        nc.vector.tensor_tensor(agg[:, :], agg[:, :], dinv128[:, :], Alu.mult)
        nc.scalar.activation(agg[:, :], agg[:, :], Act.Identity,
                             bias=b3t[:, 0:1], scale=1.0)
        for g in range(G):
            nc.sync.dma_start(out_d[0:2, g * SUB:(g + 1) * SUB],
                              agg[16 * g:16 * g + 2, 0:SUB])

    nc.compile()
    return nc


_CACHED = {}


def _in_maps(host):
    maps = []
    for c in range(CORES):
        maps.append({
            "idx_main": np.ascontiguousarray(host["idx_main_w"][c]),
            "idx_ends": np.ascontiguousarray(host["ends_w"][c]),
            "xself": np.ascontiguousarray(host["xself"][c]),
            "dinv_g": host["dinv_g"][c],
            "W1aug": host["W1aug"], "W2aug": host["W2aug"],
            "W3": host["W3"], "b3rep": host["b3rep"],
        })
    return maps


def kernel(**inputs):
    x = np.asarray(inputs["x"])
    edge_index = np.asarray(inputs["edge_index"])
    host = _preprocess(x, edge_index, inputs["W1"], inputs["b1"],
                       inputs["W2"], inputs["b2"], inputs["W3"], inputs["b3"])
    L_c = host["L_c"]
    if L_c not in _CACHED:
        _CACHED[L_c] = _build_bass(L_c)
    nc = _CACHED[L_c]

    from concourse.bass_utils import run_bass_kernel_spmd
    res = run_bass_kernel_spmd(nc, _in_maps(host), core_ids=list(range(CORES)))

    out = np.empty((N, 2), dtype=np.float32)
    P, core_of = host["P"], host["core_of"]
    for c in range(CORES):
        nodes = np.arange(c * NSH, (c + 1) * NSH)
        out[nodes] = res.results[c]["out"][:, P[nodes]].T
    return out
